# revision 12
# baseline (speedup 1.0000x reference)
"""Distributed Trainium2 Bass kernel for nn_Attention_69973607186925.

Multi-head attention (N=288 tokens, B=64 batch, C=1024, H=16 heads) with a
prompt-structured mask, data-parallel over batch across 8 NeuronCores
(8 batches = 128 heads per core, zero collectives).

Per-core dataflow (all matmuls bf16 -> f32 PSUM):
  phase A: QKV projections. q/k produced TRANSPOSED [c, token] as scores
           operands; v produced NATURAL [token, c] into 192-wide per-pair
           stationary slots [v_h0 | 1 | 0*62 | 1 | v_h1] so each head's
           PV matmul (M=128, padded) deposits BOTH the attention output
           (partition-aligned for its outT half) AND the softmax
           denominator (a spare row) in one stream -- zero extra PE work
           for the softmax sums.
  phase B: 64 head-pairs, software-pipelined so the tensor engine issues
           back-to-back (exp of pair p overlaps scores of pair p+1, PV of
           pair p-1, and proj chunks of batch b-2). PSUM staging copies run
           on the otherwise-idle GpSimd (Pool) engine; softmax reciprocal +
           broadcast (selector matmul) pipeline a batch behind.
  phase C: output projection (lag-2 batches) from outT, bias via
           per-partition tensor_scalar, DMA to DRAM [1024, 2304].

Host side: shard batch, pre-transpose/pre-cast inputs (free), gather and
re-transpose the 8 per-core outputs.
"""

import sys

if "/opt/trn_rl_repo" not in sys.path:
    sys.path.insert(0, "/opt/trn_rl_repo")

import numpy as np
import ml_dtypes

import concourse.bass as bass
import concourse.mybir as mybir
import concourse.tile as tile
from concourse.bass_utils import run_bass_kernel_spmd

BF16 = mybir.dt.bfloat16
F32 = mybir.dt.float32

N = 288          # tokens per batch
BL = 8           # batches per core
C = 1024
H = 16           # heads per batch
HD = 64          # head dim
T = BL * N       # tokens per core (2304)
CT = C // 128    # c tiles (8)
SCALE = HD ** -0.5
M_TILES = [(0, 128), (128, 128), (256, 32)]  # key tiles per batch


def _install_tile_drain_patch():
    """walrus in this container accepts only ONE semaphore wait per sync
    (SP) engine instruction; TileContext's final drain carries one wait
    per live semaphore.  Split them across single-wait nops (same engine,
    program order) before the drain."""
    from concourse.vector_clock import ScopedClock

    if getattr(tile.TileContext, "_drain_patch_installed", False):
        return

    def _drain_and_barrier_chunked(self, tick_clock, wait_clock):
        nc = self.nc
        collector = nc.sync.nop(nofuse=True, hint="drain_wait_collector")
        wait_clock.add_sem_waits(
            collector.ins, ScopedClock({None: tick_clock.global_clock})
        )
        si = collector.ins.sync_info
        waits = list(si.on_wait) if si and si.on_wait else []
        if len(waits) > 1:
            si.on_wait = waits[:1]
            for w in waits[1:]:
                extra = nc.sync.nop(nofuse=True, hint="drain_wait_chunk")
                esi = extra.ins.sync_info
                if esi is None:
                    extra.ins.sync_info = mybir.SyncInfo(on_wait=[w], on_update=[])
                else:
                    esi.on_wait = (esi.on_wait or []) + [w]
        nc.sync.drain()

        nc.all_engine_barrier()
        assert self.sems is not None
        popped = nc._tile_sem_poison_stack.pop()
        assert popped is self._sem_poison
        nc.clear_and_free_semaphores(list(self.sems.allocated().values()))
        nc.all_engine_barrier()

    tile.TileContext._drain_and_barrier = _drain_and_barrier_chunked
    tile.TileContext._drain_patch_installed = True


def _split_multi_waits(nc):
    """walrus in this container accepts only one semaphore wait per
    instruction.  For any instruction carrying N>1 waits, hoist N-1 of
    them onto same-engine NoOps placed immediately before it — engine
    program order makes this equivalent."""
    for fn in nc.m.functions:
        for blk in fn.blocks:
            insts = blk.instructions
            out = []
            changed = False
            for inst in insts:
                si = inst.sync_info
                if si is not None and si.on_wait and len(si.on_wait) > 1:
                    waits = list(si.on_wait)
                    for idx, w in enumerate(waits[:-1]):
                        out.append(
                            mybir.InstNoOp(
                                name=f"{inst.name}-hw{idx}",
                                engine=inst.engine,
                                ins=[],
                                outs=[],
                                bass_nofuse=True,
                                sync_info=mybir.SyncInfo(on_wait=[w], on_update=[]),
                            )
                        )
                    si.on_wait = [waits[-1]]
                    changed = True
                out.append(inst)
            if changed:
                insts[:] = out


def _build_nc(split_waits=True):
    _install_tile_drain_patch()
    nc = bass.Bass()

    xt_ext = nc.declare_dram_parameter("xt", [C, T], BF16, isOutput=False)
    wqkt_ext = nc.declare_dram_parameter("wqkt", [C, 2 * C], BF16, isOutput=False)
    wvt_ext = nc.declare_dram_parameter("wvt", [C, C], BF16, isOutput=False)
    wpt_ext = nc.declare_dram_parameter("wpt", [C, C], BF16, isOutput=False)
    bv_ext = nc.declare_dram_parameter("bv", [1, C], BF16, isOutput=False)
    bqk_ext = nc.declare_dram_parameter("bqk", [128, 16], F32, isOutput=False)
    bp_ext = nc.declare_dram_parameter("bp", [128, CT], F32, isOutput=False)
    mask_ext = nc.declare_dram_parameter("binmask", [32, N], BF16, isOutput=False)
    sel2_ext = nc.declare_dram_parameter("sel2", [2, 128], BF16, isOutput=False)
    out_ext = nc.declare_dram_parameter("out", [C, T], F32, isOutput=True)

    xt_r = xt_ext.rearrange("(o p) t -> p o t", p=128)
    wqkt_r = wqkt_ext.rearrange("(o p) j -> p o j", p=128)
    wvt_r = wvt_ext.rearrange("(o p) j -> p o j", p=128)
    wpt_r = wpt_ext.rearrange("(o p) j -> p o j", p=128)
    out_r = out_ext.rearrange("(o p) t -> p o t", p=128)

    with tile.TileContext(nc) as tc:
        with (
            tc.tile_pool(name="persist", bufs=1) as persist,
            tc.tile_pool(name="consts", bufs=1) as consts,
        ):
            qt_sb = persist.tile([128, CT, T], BF16, tag="qt")
            kt_sb = persist.tile([128, CT, T], BF16, tag="kt")
            # per-pair 192-wide PV stationary slots: [v_even |1| 0*62 |1| v_odd]
            vaug_sb = persist.tile([128, BL, 2, 8, 192], BF16, tag="vaug")
            v2aug_sb = persist.tile([128, 2, 8, 192], BF16, tag="v2aug")

            bqk_sb = consts.tile([128, 16], F32, tag="bqk")
            bp_sb = consts.tile([128, CT], F32, tag="bp")
            bv_sb = consts.tile([1, C], BF16, tag="bv")
            mask_sb = consts.tile([32, N], BF16, tag="binmask")
            zbias_sb = consts.tile([128, 1], F32, tag="zbias")
            sel2_sb = consts.tile([2, 128], BF16, tag="sel2")
            onesr_sb = consts.tile([1, 128], BF16, tag="onesr")
            nc.sync.dma_start(out=bqk_sb[:], in_=bqk_ext[:])
            nc.sync.dma_start(out=bp_sb[:], in_=bp_ext[:])
            nc.sync.dma_start(out=bv_sb[:], in_=bv_ext[:])
            nc.sync.dma_start(out=mask_sb[:], in_=mask_ext[:])
            nc.sync.dma_start(out=sel2_sb[:], in_=sel2_ext[:])
            nc.vector.memset(zbias_sb[:], 0.0)
            nc.vector.memset(onesr_sb[:], 1.0)
            # ones / zero padding in the PV stationary slots
            nc.vector.memset(vaug_sb[:, :, :, :, 64:65], 1.0)
            nc.vector.memset(vaug_sb[:, :, :, :, 127:128], 1.0)
            nc.vector.memset(vaug_sb[:, :, :, :, 65:127], 0.0)
            nc.vector.memset(v2aug_sb[:, :, :, 64:65], 1.0)
            nc.vector.memset(v2aug_sb[:, :, :, 127:128], 1.0)
            nc.vector.memset(v2aug_sb[:, :, :, 65:127], 0.0)

            # ---------------- phase A: QKV projections ----------------
            with (
                tc.tile_pool(name="xa", bufs=1) as xa_pool,
                tc.tile_pool(name="wa", bufs=2) as wa_pool,
                tc.tile_pool(name="psA", bufs=4, space="PSUM") as psa_pool,
                tc.tile_pool(name="psAv", bufs=2, space="PSUM") as psav_pool,
            ):
                xt_sb = xa_pool.tile([128, CT, T], BF16, tag="xt")
                # chunk-major x load so the first q matmuls start early
                for c0 in range(0, T, 512):
                    csz = min(512, T - c0)
                    for o in range(CT):
                        nc.sync.dma_start(
                            out=xt_sb[:, o, c0 : c0 + csz],
                            in_=xt_r[:, o, c0 : c0 + csz],
                        )

                # q then k, transposed layout [cq, t]
                for proj in range(2):
                    dst = qt_sb if proj == 0 else kt_sb
                    for o in range(CT):
                        w_sb = wa_pool.tile([128, CT, 128], BF16, tag="wqk")
                        j0 = proj * C + o * 128
                        nc.sync.dma_start(
                            out=w_sb[:], in_=wqkt_r[:, :, j0 : j0 + 128]
                        )
                        for c0 in range(0, T, 512):
                            csz = min(512, T - c0)
                            ps = psa_pool.tile([128, 512], F32, tag="psqk")
                            for kk in range(CT):
                                nc.tensor.matmul(
                                    ps[:, 0:csz],
                                    lhsT=w_sb[:, kk, :],
                                    rhs=xt_sb[:, kk, c0 : c0 + csz],
                                    start=(kk == 0),
                                    stop=(kk == CT - 1),
                                )
                            nc.vector.tensor_scalar(
                                out=dst[:, o, c0 : c0 + csz],
                                in0=ps[:, 0:csz],
                                scalar1=bqk_sb[:, proj * 8 + o : proj * 8 + o + 1],
                                scalar2=None,
                                op0=mybir.AluOpType.add,
                            )

                # contiguous staging of the 32-token mt2 tails, 4 batches
                # per 128-wide group (walrus: stationary AP needs 1 free dim)
                xg2_sb = xa_pool.tile([128, CT, 2, 128], BF16, tag="xg2")
                for kk in range(CT):
                    for g in range(2):
                        nc.vector.tensor_copy(
                            xg2_sb[:, kk, g, :],
                            xt_sb[:, kk, :].rearrange("p (b n) -> p b n", n=N)[
                                :, 4 * g : 4 * g + 4, 256:288
                            ],
                        )

                # v, natural layout [token, c] into the 192-wide pair slots
                for ch in range(2):
                    p0 = 4 * ch  # pair slots covered by this channel half
                    wv_sb = wa_pool.tile([128, CT, 512], BF16, tag="wv")
                    nc.sync.dma_start(
                        out=wv_sb[:], in_=wvt_r[:, :, ch * 512 : (ch + 1) * 512]
                    )
                    for b in range(BL):
                        for mt, (moff, msize) in enumerate(M_TILES[:2]):
                            t0 = b * N + moff
                            ps = psav_pool.tile([128, 512], F32, tag="psv")
                            for kk in range(CT):
                                nc.tensor.matmul(
                                    ps[:msize, :],
                                    lhsT=xt_sb[:, kk, t0 : t0 + msize],
                                    rhs=wv_sb[:, kk, :],
                                    start=(kk == 0),
                                    stop=False,
                                )
                            # bias row via rank-1 matmul (ones ⊗ bv)
                            nc.tensor.matmul(
                                ps[:msize, :],
                                lhsT=onesr_sb[0:1, 0:msize],
                                rhs=bv_sb[0:1, ch * 512 : (ch + 1) * 512],
                                start=False,
                                stop=True,
                            )
                            psr = ps[:msize, :].rearrange(
                                "m (h s c) -> m h s c", s=2, c=64
                            )
                            nc.scalar.copy(
                                out=vaug_sb[0:msize, b, mt, p0 : p0 + 4, 0:64],
                                in_=psr[:, :, 0, :],
                            )
                            nc.scalar.copy(
                                out=vaug_sb[0:msize, b, mt, p0 : p0 + 4, 128:192],
                                in_=psr[:, :, 1, :],
                            )
                    # mt2 (32-token tails): 4 batches packed on partitions
                    for g in range(2):
                        ps = psav_pool.tile([128, 512], F32, tag="psv")
                        for kk in range(CT):
                            nc.tensor.matmul(
                                ps[:],
                                lhsT=xg2_sb[:, kk, g, :],
                                rhs=wv_sb[:, kk, :],
                                start=(kk == 0),
                                stop=False,
                            )
                        nc.tensor.matmul(
                            ps[:],
                            lhsT=onesr_sb[0:1, 0:128],
                            rhs=bv_sb[0:1, ch * 512 : (ch + 1) * 512],
                            start=False,
                            stop=True,
                        )
                        for jj in range(4):
                            psr = ps[32 * jj : 32 * jj + 32, :].rearrange(
                                "m (h s c) -> m h s c", s=2, c=64
                            )
                            nc.scalar.copy(
                                out=v2aug_sb[
                                    32 * jj : 32 * jj + 32, g, p0 : p0 + 4, 0:64
                                ],
                                in_=psr[:, :, 0, :],
                            )
                            nc.scalar.copy(
                                out=v2aug_sb[
                                    32 * jj : 32 * jj + 32, g, p0 : p0 + 4, 128:192
                                ],
                                in_=psr[:, :, 1, :],
                            )

            # ---------------- phases B+C: pipelined attention ----------------
            with (
                tc.tile_pool(name="wpt", bufs=1) as wpt_pool,
                tc.tile_pool(name="outt", bufs=3) as outt_pool,
                tc.tile_pool(name="pvst", bufs=2) as pvst_pool,
                tc.tile_pool(name="dense", bufs=2) as dense_pool,
                tc.tile_pool(name="dp", bufs=2) as dp_pool,
                tc.tile_pool(name="yc", bufs=2) as yc_pool,
                tc.tile_pool(name="expt", bufs=2) as expt_pool,
                tc.tile_pool(name="psS", bufs=2, space="PSUM") as pss_pool,
                tc.tile_pool(name="psPV", bufs=1, space="PSUM") as pspv_pool,
                tc.tile_pool(name="psC", bufs=1, space="PSUM") as psc_pool,
                tc.tile_pool(name="psBC", bufs=1, space="PSUM") as psbc_pool,
            ):
                wpt_sb = wpt_pool.tile([128, CT, C], BF16, tag="wpt")
                for kk in range(CT):
                    nc.sync.dma_start(out=wpt_sb[:, kk, :], in_=wpt_r[:, kk, :])

                # pipeline state
                ps_ss = {}      # gp -> scores PSUM tile
                expts = {}      # gp -> [expt_mt0, expt_mt1, expt_mt2]
                pvs = {}        # gp -> PV PSUM tile
                outts = {}      # b -> outT tile
                pvstAs = {}     # b -> even-head pv staging (+sum row 64)
                pvstBs = {}     # b -> odd-head pv staging (+sum row 63)
                denses = {}     # b -> dense sums tile
                densebs = {}    # b -> bf16 reciprocal tile
                proj_ps = {}    # (b, o) -> proj PSUM tile

                def emit_scores(gp, mt):
                    b, p = divmod(gp, 8)
                    o = p
                    moff, msize = M_TILES[mt]
                    mb = (b % 4) * 32 if mt == 2 else 0
                    if mt == 0:
                        ps_ss[gp] = pss_pool.tile(
                            [128, 2, 512], F32, tag="ps_s", name="ps_s"
                        )
                        expts[gp] = [
                            expt_pool.tile(
                                [128, 2, N], BF16, tag=f"expt{m}", name=f"expt{m}"
                            )
                            for m in range(3)
                        ]
                    ps_s = ps_ss[gp]
                    for hh in range(2):
                        rb = 64 * hh
                        nc.tensor.matmul(
                            ps_s[mb : mb + msize, hh, 0:N],
                            lhsT=kt_sb[
                                rb : rb + 64, o, b * N + moff : b * N + moff + msize
                            ],
                            rhs=qt_sb[rb : rb + 64, o, b * N : (b + 1) * N],
                            start=True,
                            stop=True,
                            tile_position=(rb, mb) if mt == 2 else None,
                        )

                def emit_exp(gp, mt):
                    b, p = divmod(gp, 8)
                    moff, msize = M_TILES[mt]
                    mb = (b % 4) * 32 if mt == 2 else 0
                    nc.scalar.activation(
                        out=expts[gp][mt][mb : mb + msize, 0:2, :],
                        in_=ps_ss[gp][mb : mb + msize, :, 0:N],
                        func=mybir.ActivationFunctionType.Exp,
                        bias=zbias_sb[0:msize, 0:1],
                        scale=SCALE,
                    )

                def emit_mask(gp):
                    e0 = expts[gp][0]
                    nc.gpsimd.tensor_tensor(
                        e0[0:32, 0:2, :],
                        e0[0:32, 0:2, :],
                        mask_sb[:, None, :].to_broadcast((32, 2, N)),
                        mybir.AluOpType.mult,
                    )

                def emit_pv(gp, mt):
                    b, p = divmod(gp, 8)
                    moff, msize = M_TILES[mt]
                    mb = (b % 4) * 32 if mt == 2 else 0
                    if mt == 0:
                        pvs[gp] = pspv_pool.tile(
                            [128, 2, 512], F32, tag="ps_pv", name="ps_pv"
                        )
                    ps_pv = pvs[gp]
                    for hh in range(2):
                        if mt < 2:
                            lhsT_v = vaug_sb[
                                0:msize, b, mt, p, 64 * hh : 64 * hh + 128
                            ]
                        else:
                            lhsT_v = v2aug_sb[
                                mb : mb + 32, b // 4, p, 64 * hh : 64 * hh + 128
                            ]
                        nc.tensor.matmul(
                            ps_pv[0:128, hh, 0:N],
                            lhsT=lhsT_v,
                            rhs=expts[gp][mt][mb : mb + msize, hh, :],
                            start=(mt == 0),
                            stop=(mt == 2),
                            skip_group_check=True,
                            tile_position=(mb, 0) if mt == 2 else None,
                        )

                def emit_pv_drain(gp):
                    # PV rows out of PSUM (frees the banks); the softmax-sum
                    # rows (bank0 row 64 / bank1 row 63) ride along, then
                    # SBUF->SBUF DMAs compact them into dense.
                    b, p = divmod(gp, 8)
                    ps_pv = pvs.pop(gp)
                    pvA = pvstAs[b]
                    pvB = pvstBs[b]
                    nc.vector.tensor_copy(pvA[0:65, p, :], ps_pv[0:65, 0, 0:N])
                    nc.vector.tensor_copy(pvB[0:128, p, :], ps_pv[0:128, 1, 0:N])
                    dense = denses[b]
                    nc.sync.dma_start(
                        out=dense[2 * p : 2 * p + 1, :], in_=pvA[64:65, p, :]
                    )
                    nc.sync.dma_start(
                        out=dense[2 * p + 1 : 2 * p + 2, :],
                        in_=pvB[63:64, p, :],
                    )

                def emit_recip(b):
                    dense = denses[b]
                    densef = dense_pool.tile(
                        [16, N], F32, tag="densef", name="densef"
                    )
                    denseb = dense_pool.tile(
                        [16, N], BF16, tag="denseb", name="denseb"
                    )
                    densebs[b] = denseb
                    nc.vector.tensor_copy(densef[:], dense[:])
                    nc.vector.reciprocal(out=densef[:], in_=densef[:])
                    nc.vector.tensor_copy(denseb[:], densef[:])

                def emit_bcast_norm(b, o):
                    denseb = densebs[b]
                    dp = dp_pool.tile([2, N], BF16, tag="dp", name="dp")
                    nc.sync.dma_start(out=dp[:], in_=denseb[2 * o : 2 * o + 2, :])
                    psbc = psbc_pool.tile([128, N], F32, tag="psbc", name="psbc")
                    nc.tensor.matmul(
                        psbc[:], lhsT=sel2_sb[:], rhs=dp[:], start=True, stop=True
                    )
                    nc.vector.tensor_tensor(
                        outts[b][0:64, o, :],
                        pvstAs[b][0:64, o, :],
                        psbc[0:64, :],
                        mybir.AluOpType.mult,
                    )
                    nc.vector.tensor_tensor(
                        outts[b][64:128, o, :],
                        pvstBs[b][64:128, o, :],
                        psbc[64:128, :],
                        mybir.AluOpType.mult,
                    )

                def emit_proj(b, o, kks):
                    if kks[0] == 0:
                        proj_ps[(b, o)] = psc_pool.tile(
                            [128, N], F32, tag="psy", name="psy"
                        )
                    ps = proj_ps[(b, o)]
                    for kk in kks:
                        nc.tensor.matmul(
                            ps[:],
                            lhsT=wpt_sb[:, kk, o * 128 : (o + 1) * 128],
                            rhs=outts[b][:, kk, :],
                            start=(kk == 0),
                            stop=(kk == CT - 1),
                        )

                def emit_proj_finish(b, o):
                    ps = proj_ps.pop((b, o))
                    y_sb = yc_pool.tile([128, N], F32, tag="y", name="y")
                    nc.vector.tensor_scalar(
                        out=y_sb[:],
                        in0=ps[:],
                        scalar1=bp_sb[:, o : o + 1],
                        scalar2=None,
                        op0=mybir.AluOpType.add,
                    )
                    nc.sync.dma_start(
                        out=out_r[:, o, b * N : (b + 1) * N], in_=y_sb[:]
                    )

                def new_batch(b):
                    outts[b] = outt_pool.tile(
                        [128, CT, N], BF16, tag="outt", name="outt"
                    )
                    pvstAs[b] = pvst_pool.tile(
                        [128, 8, N], BF16, tag="pvstA", name="pvstA"
                    )
                    pvstBs[b] = pvst_pool.tile(
                        [128, 8, N], BF16, tag="pvstB", name="pvstB"
                    )
                    denses[b] = dense_pool.tile(
                        [16, N], BF16, tag="dense", name="dense"
                    )

                # ---------------- main pipelined loop ----------------
                for gp in range(64):
                    b, p = divmod(gp, 8)
                    if p == 0:
                        new_batch(b)
                    # A
                    emit_scores(gp, 0)
                    emit_exp(gp, 0)
                    emit_mask(gp)
                    # B
                    if gp >= 1:
                        emit_pv(gp - 1, 0)
                        emit_pv(gp - 1, 1)
                    # C
                    if b >= 2:
                        emit_proj(b - 2, p, [0, 1, 2])
                    # D
                    emit_scores(gp, 1)
                    emit_exp(gp, 1)
                    # E
                    if gp >= 1:
                        emit_pv(gp - 1, 2)
                        emit_pv_drain(gp - 1)
                    if p == 0 and b >= 1:
                        emit_recip(b - 1)
                    # F
                    if b >= 2:
                        emit_proj(b - 2, p, [3, 4, 5, 6])
                    if b >= 1 and p >= 1:
                        emit_bcast_norm(b - 1, p - 1)
                    # G
                    emit_scores(gp, 2)
                    emit_exp(gp, 2)
                    # H
                    if b >= 2:
                        emit_proj(b - 2, p, [7])
                        emit_proj_finish(b - 2, p)
                    if b >= 1 and p == 7:
                        emit_bcast_norm(b - 1, 7)

                # ---------------- epilogue ----------------
                emit_pv(63, 0)
                emit_pv(63, 1)
                emit_pv(63, 2)
                emit_pv_drain(63)
                emit_recip(7)
                for o in range(CT):
                    emit_proj(6, o, [0, 1, 2, 3])
                    if o >= 1:
                        emit_bcast_norm(7, o - 1)
                    emit_proj(6, o, [4, 5, 6, 7])
                    emit_proj_finish(6, o)
                emit_bcast_norm(7, 7)
                for o in range(CT):
                    emit_proj(7, o, [0, 1, 2, 3, 4, 5, 6, 7])
                    emit_proj_finish(7, o)

    if split_waits:
        _split_multi_waits(nc)
    return nc


_NC_CACHE = None


def _get_nc():
    global _NC_CACHE
    if _NC_CACHE is None:
        _NC_CACHE = _build_nc()
    return _NC_CACHE


def _host_inputs(x, Wqkv, bqkv, Wproj, bproj):
    bf16 = ml_dtypes.bfloat16
    shared = {}
    shared["wqkt"] = np.ascontiguousarray(Wqkv[: 2 * C].T).astype(bf16)
    shared["wvt"] = np.ascontiguousarray(Wqkv[2 * C :].T).astype(bf16)
    shared["wpt"] = np.ascontiguousarray(Wproj.T).astype(bf16)
    shared["bv"] = bqkv[2 * C :].reshape(1, C).astype(bf16)
    shared["bqk"] = np.ascontiguousarray(
        bqkv[: 2 * C].reshape(2, 8, 128).transpose(2, 0, 1).reshape(128, 16)
    ).astype(np.float32)
    shared["bp"] = np.ascontiguousarray(bproj.reshape(CT, 128).T).astype(np.float32)
    m_ = np.arange(32)[:, None]
    n_ = np.arange(N)[None, :]
    shared["binmask"] = ((n_ < 32) & (n_ >= 4 * (m_ // 4))).astype(bf16)
    sel2 = np.zeros((2, 128), bf16)
    sel2[0, 0:64] = 1.0
    sel2[1, 64:128] = 1.0
    shared["sel2"] = sel2

    in_maps = []
    for i in range(8):
        xc = x[:, i * BL : (i + 1) * BL, :]  # (N, BL, C)
        xt = np.ascontiguousarray(xc.transpose(2, 1, 0).reshape(C, T)).astype(bf16)
        m = dict(shared)
        m["xt"] = xt
        in_maps.append(m)
    return in_maps


def kernel(x, Wqkv, bqkv, Wproj, bproj):
    x = np.asarray(x, dtype=np.float32)
    Wqkv = np.asarray(Wqkv, dtype=np.float32)
    bqkv = np.asarray(bqkv, dtype=np.float32)
    Wproj = np.asarray(Wproj, dtype=np.float32)
    bproj = np.asarray(bproj, dtype=np.float32)

    nc = _get_nc()
    in_maps = _host_inputs(x, Wqkv, bqkv, Wproj, bproj)
    res = run_bass_kernel_spmd(nc, in_maps, core_ids=list(range(8)))

    full = np.empty((N, 64, C), dtype=np.float32)
    for i in range(8):
        yT = np.asarray(res.results[i]["out"], dtype=np.float32)  # [C, T]
        full[:, i * BL : (i + 1) * BL, :] = yT.reshape(C, BL, N).transpose(2, 1, 0)
    return full


# revision 30
# speedup vs baseline: 1.0657x; 1.0657x over previous
"""Distributed Trainium2 Bass kernel for nn_Attention_69973607186925.

Multi-head attention (N=288 tokens, B=64 batch, C=1024, H=16 heads) with a
prompt-structured mask, data-parallel over batch across 8 NeuronCores
(8 batches = 128 heads per core, zero collectives).

Per-core dataflow (all matmuls bf16 -> f32 PSUM):
  phase A: QKV projections. q/k produced TRANSPOSED [c, token] as scores
           operands; v produced NATURAL [token, c] into 192-wide per-pair
           stationary slots [v_h0 | 1 | 0*62 | 1 | v_h1] so each head's
           PV matmul (M=128, padded) deposits BOTH the attention output
           (partition-aligned for its outT half) AND the softmax
           denominator (a spare row) in one stream -- zero extra PE work
           for the softmax sums.
  phase B: 64 head-pairs, software-pipelined so the tensor engine issues
           back-to-back (exp of pair p overlaps scores of pair p+1, PV of
           pair p-1, and proj chunks of batch b-2). PSUM staging copies run
           on the otherwise-idle GpSimd (Pool) engine; softmax reciprocal +
           broadcast (selector matmul) pipeline a batch behind.
  phase C: output projection (lag-2 batches) from outT, bias via
           per-partition tensor_scalar, DMA to DRAM [1024, 2304].

Host side: shard batch, pre-transpose/pre-cast inputs (free), gather and
re-transpose the 8 per-core outputs.
"""

import sys

if "/opt/trn_rl_repo" not in sys.path:
    sys.path.insert(0, "/opt/trn_rl_repo")

import numpy as np
import ml_dtypes

import concourse.bass as bass
import concourse.mybir as mybir
import concourse.tile as tile
from concourse.bass_utils import run_bass_kernel_spmd

BF16 = mybir.dt.bfloat16
F32 = mybir.dt.float32

N = 288          # tokens per batch
BL = 8           # batches per core
C = 1024
H = 16           # heads per batch
HD = 64          # head dim
T = BL * N       # tokens per core (2304)
CT = C // 128    # c tiles (8)
SCALE = HD ** -0.5
M_TILES = [(0, 128), (128, 128), (256, 32)]  # key tiles per batch


def _install_tile_drain_patch():
    """walrus in this container accepts only ONE semaphore wait per sync
    (SP) engine instruction; TileContext's final drain carries one wait
    per live semaphore.  Split them across single-wait nops (same engine,
    program order) before the drain."""
    from concourse.vector_clock import ScopedClock

    if getattr(tile.TileContext, "_drain_patch_installed", False):
        return

    def _drain_and_barrier_chunked(self, tick_clock, wait_clock):
        nc = self.nc
        collector = nc.sync.nop(nofuse=True, hint="drain_wait_collector")
        wait_clock.add_sem_waits(
            collector.ins, ScopedClock({None: tick_clock.global_clock})
        )
        si = collector.ins.sync_info
        waits = list(si.on_wait) if si and si.on_wait else []
        if len(waits) > 1:
            si.on_wait = waits[:1]
            for w in waits[1:]:
                extra = nc.sync.nop(nofuse=True, hint="drain_wait_chunk")
                esi = extra.ins.sync_info
                if esi is None:
                    extra.ins.sync_info = mybir.SyncInfo(on_wait=[w], on_update=[])
                else:
                    esi.on_wait = (esi.on_wait or []) + [w]
        nc.sync.drain()

        nc.all_engine_barrier()
        assert self.sems is not None
        popped = nc._tile_sem_poison_stack.pop()
        assert popped is self._sem_poison
        nc.clear_and_free_semaphores(list(self.sems.allocated().values()))
        nc.all_engine_barrier()

    tile.TileContext._drain_and_barrier = _drain_and_barrier_chunked
    tile.TileContext._drain_patch_installed = True


def _split_multi_waits(nc):
    """walrus in this container accepts only one semaphore wait per
    instruction.  For any instruction carrying N>1 waits, hoist N-1 of
    them onto same-engine NoOps placed immediately before it — engine
    program order makes this equivalent."""
    for fn in nc.m.functions:
        for blk in fn.blocks:
            insts = blk.instructions
            out = []
            changed = False
            for inst in insts:
                si = inst.sync_info
                if si is not None and si.on_wait and len(si.on_wait) > 1:
                    waits = list(si.on_wait)
                    for idx, w in enumerate(waits[:-1]):
                        out.append(
                            mybir.InstNoOp(
                                name=f"{inst.name}-hw{idx}",
                                engine=inst.engine,
                                ins=[],
                                outs=[],
                                bass_nofuse=True,
                                sync_info=mybir.SyncInfo(on_wait=[w], on_update=[]),
                            )
                        )
                    si.on_wait = [waits[-1]]
                    changed = True
                out.append(inst)
            if changed:
                insts[:] = out


def _build_nc(split_waits=True):
    _install_tile_drain_patch()
    nc = bass.Bass()

    xt_ext = nc.declare_dram_parameter("xt", [C, T], BF16, isOutput=False)
    wqkt_ext = nc.declare_dram_parameter("wqkt", [C, 2 * C], BF16, isOutput=False)
    wvt_ext = nc.declare_dram_parameter("wvt", [C, C], BF16, isOutput=False)
    wpt_ext = nc.declare_dram_parameter("wpt", [C, C], BF16, isOutput=False)
    bv_ext = nc.declare_dram_parameter("bv", [1, C], BF16, isOutput=False)
    bqk_ext = nc.declare_dram_parameter("bqk", [128, 16], F32, isOutput=False)
    bp_ext = nc.declare_dram_parameter("bp", [128, CT], F32, isOutput=False)
    mask_ext = nc.declare_dram_parameter("binmask", [32, N], BF16, isOutput=False)
    sel2_ext = nc.declare_dram_parameter("sel2", [2, 128], BF16, isOutput=False)
    out_ext = nc.declare_dram_parameter("out", [C, T], F32, isOutput=True)

    xt_r = xt_ext.rearrange("(o p) t -> p o t", p=128)
    wqkt_r = wqkt_ext.rearrange("(o p) j -> p o j", p=128)
    wvt_r = wvt_ext.rearrange("(o p) j -> p o j", p=128)
    wpt_r = wpt_ext.rearrange("(o p) j -> p o j", p=128)
    out_r = out_ext.rearrange("(o p) t -> p o t", p=128)

    with tile.TileContext(nc) as tc:
        with (
            tc.tile_pool(name="persist", bufs=1) as persist,
            tc.tile_pool(name="consts", bufs=1) as consts,
        ):
            qt_sb = persist.tile([128, CT, T], BF16, tag="qt")
            kt_sb = persist.tile([128, CT, T], BF16, tag="kt")
            # per-pair 192-wide PV stationary slots: [v_even |1| 0*62 |1| v_odd]
            vaug_sb = persist.tile([128, BL, 2, 8, 192], BF16, tag="vaug")
            v2aug_sb = persist.tile([128, 2, 8, 192], BF16, tag="v2aug")

            bqk_sb = consts.tile([128, 16], F32, tag="bqk")
            bp_sb = consts.tile([128, CT], F32, tag="bp")
            bv_sb = consts.tile([1, C], BF16, tag="bv")
            mask_sb = consts.tile([32, N], BF16, tag="binmask")
            zbias_sb = consts.tile([128, 1], F32, tag="zbias")
            sel2_sb = consts.tile([2, 128], BF16, tag="sel2")
            onesr_sb = consts.tile([1, 128], BF16, tag="onesr")
            nc.vector.memset(zbias_sb[:], 0.0)
            nc.vector.memset(onesr_sb[:], 1.0)
            # ones / zero padding in the PV stationary slots
            nc.vector.memset(vaug_sb[:, :, :, :, 64:65], 1.0)
            nc.vector.memset(vaug_sb[:, :, :, :, 127:128], 1.0)
            nc.vector.memset(vaug_sb[:, :, :, :, 65:127], 0.0)
            nc.vector.memset(v2aug_sb[:, :, :, 64:65], 1.0)
            nc.vector.memset(v2aug_sb[:, :, :, 127:128], 1.0)
            nc.vector.memset(v2aug_sb[:, :, :, 65:127], 0.0)

            # ---------------- phase A: QKV projections ----------------
            with (
                tc.tile_pool(name="xa", bufs=1) as xa_pool,
                tc.tile_pool(name="wa", bufs=2) as wa_pool,
                tc.tile_pool(name="psA", bufs=4, space="PSUM") as psa_pool,
                tc.tile_pool(name="psAv", bufs=2, space="PSUM") as psav_pool,
            ):
                xt_sb = xa_pool.tile([128, CT, T], BF16, tag="xt")
                # Startup choreography: each dma_start blocks its issuing
                # engine ~0.6us, so spread dispatch across the two HWDGE
                # queues (sync + scalar) and order so the first q matmul's
                # operands (w_q[o=0] + x chunk0) land first.
                # w_q[o=0] first, split by kk halves across both queues
                w0_sb = wa_pool.tile([128, CT, 128], BF16, tag="wqk", name="w0_sb")
                nc.sync.dma_start(out=w0_sb[:, 0:4, :], in_=wqkt_r[:, 0:4, 0:128])
                nc.scalar.dma_start(out=w0_sb[:, 4:8, :], in_=wqkt_r[:, 4:8, 0:128])
                # x chunk0 split over both queues; later chunks sync-only so
                # the scalar queue stays free for the rolling w prefetch
                X_CHUNKS = [(0, 512), (512, 896), (1408, 896)]
                for ci, (c0, csz) in enumerate(X_CHUNKS):
                    for o in range(CT):
                        eng = nc.sync if (ci > 0 or o % 2 == 0) else nc.scalar
                        eng.dma_start(
                            out=xt_sb[:, o, c0 : c0 + csz],
                            in_=xt_r[:, o, c0 : c0 + csz],
                        )
                nc.sync.dma_start(out=bqk_sb[:], in_=bqk_ext[:])
                nc.sync.dma_start(out=bp_sb[:], in_=bp_ext[:])
                nc.sync.dma_start(out=bv_sb[:], in_=bv_ext[:])
                nc.sync.dma_start(out=mask_sb[:], in_=mask_ext[:])
                nc.sync.dma_start(out=sel2_sb[:], in_=sel2_ext[:])

                # q then k, transposed layout [cq, t]
                for proj in range(2):
                    dst = qt_sb if proj == 0 else kt_sb
                    for o in range(CT):
                        if proj == 0 and o == 0:
                            w_sb = w0_sb
                        else:
                            w_sb = wa_pool.tile(
                                [128, CT, 128], BF16, tag="wqk", name="w_sb"
                            )
                            j0 = proj * C + o * 128
                            nc.scalar.dma_start(
                                out=w_sb[:], in_=wqkt_r[:, :, j0 : j0 + 128]
                            )
                        for c0 in range(0, T, 512):
                            csz = min(512, T - c0)
                            ps = psa_pool.tile([128, 512], F32, tag="psqk")
                            for kk in range(CT):
                                nc.tensor.matmul(
                                    ps[:, 0:csz],
                                    lhsT=w_sb[:, kk, :],
                                    rhs=xt_sb[:, kk, c0 : c0 + csz],
                                    start=(kk == 0),
                                    stop=(kk == CT - 1),
                                )
                            nc.vector.tensor_scalar(
                                out=dst[:, o, c0 : c0 + csz],
                                in0=ps[:, 0:csz],
                                scalar1=bqk_sb[:, proj * 8 + o : proj * 8 + o + 1],
                                scalar2=None,
                                op0=mybir.AluOpType.add,
                            )

                # contiguous staging of the 32-token mt2 tails, 4 batches
                # per 128-wide group (walrus: stationary AP needs 1 free dim)
                xg2_sb = xa_pool.tile([128, CT, 2, 128], BF16, tag="xg2")
                for kk in range(CT):
                    for g in range(2):
                        nc.vector.tensor_copy(
                            xg2_sb[:, kk, g, :],
                            xt_sb[:, kk, :].rearrange("p (b n) -> p b n", n=N)[
                                :, 4 * g : 4 * g + 4, 256:288
                            ],
                        )

                # v, natural layout [token, c] into the 192-wide pair slots
                wv_sbs = []
                for ch in range(2):
                    wv_sb = wa_pool.tile([128, CT, 512], BF16, tag="wv")
                    nc.scalar.dma_start(
                        out=wv_sb[:], in_=wvt_r[:, :, ch * 512 : (ch + 1) * 512]
                    )
                    wv_sbs.append(wv_sb)
                for ch in range(2):
                    p0 = 4 * ch  # pair slots covered by this channel half
                    wv_sb = wv_sbs[ch]
                    for b in range(BL):
                        for mt, (moff, msize) in enumerate(M_TILES[:2]):
                            t0 = b * N + moff
                            ps = psav_pool.tile([128, 512], F32, tag="psv")
                            for kk in range(CT):
                                nc.tensor.matmul(
                                    ps[:msize, :],
                                    lhsT=xt_sb[:, kk, t0 : t0 + msize],
                                    rhs=wv_sb[:, kk, :],
                                    start=(kk == 0),
                                    stop=False,
                                )
                            # bias row via rank-1 matmul (ones ⊗ bv)
                            nc.tensor.matmul(
                                ps[:msize, :],
                                lhsT=onesr_sb[0:1, 0:msize],
                                rhs=bv_sb[0:1, ch * 512 : (ch + 1) * 512],
                                start=False,
                                stop=True,
                            )
                            psr = ps[:msize, :].rearrange(
                                "m (h s c) -> m h s c", s=2, c=64
                            )
                            nc.scalar.copy(
                                out=vaug_sb[0:msize, b, mt, p0 : p0 + 4, 0:64],
                                in_=psr[:, :, 0, :],
                            )
                            nc.scalar.copy(
                                out=vaug_sb[0:msize, b, mt, p0 : p0 + 4, 128:192],
                                in_=psr[:, :, 1, :],
                            )
                    # mt2 (32-token tails): 4 batches packed on partitions
                    for g in range(2):
                        ps = psav_pool.tile([128, 512], F32, tag="psv")
                        for kk in range(CT):
                            nc.tensor.matmul(
                                ps[:],
                                lhsT=xg2_sb[:, kk, g, :],
                                rhs=wv_sb[:, kk, :],
                                start=(kk == 0),
                                stop=False,
                            )
                        nc.tensor.matmul(
                            ps[:],
                            lhsT=onesr_sb[0:1, 0:128],
                            rhs=bv_sb[0:1, ch * 512 : (ch + 1) * 512],
                            start=False,
                            stop=True,
                        )
                        for jj in range(4):
                            psr = ps[32 * jj : 32 * jj + 32, :].rearrange(
                                "m (h s c) -> m h s c", s=2, c=64
                            )
                            nc.scalar.copy(
                                out=v2aug_sb[
                                    32 * jj : 32 * jj + 32, g, p0 : p0 + 4, 0:64
                                ],
                                in_=psr[:, :, 0, :],
                            )
                            nc.scalar.copy(
                                out=v2aug_sb[
                                    32 * jj : 32 * jj + 32, g, p0 : p0 + 4, 128:192
                                ],
                                in_=psr[:, :, 1, :],
                            )

            # ---------------- phases B+C: pipelined attention ----------------
            with (
                tc.tile_pool(name="wpt", bufs=1) as wpt_pool,
                tc.tile_pool(name="outt", bufs=3) as outt_pool,
                tc.tile_pool(name="pvst", bufs=2) as pvst_pool,
                tc.tile_pool(name="dense", bufs=2) as dense_pool,
                tc.tile_pool(name="dp", bufs=2) as dp_pool,
                tc.tile_pool(name="yc", bufs=2) as yc_pool,
                tc.tile_pool(name="expt", bufs=2) as expt_pool,
                tc.tile_pool(name="psS", bufs=2, space="PSUM") as pss_pool,
                tc.tile_pool(name="psPV", bufs=1, space="PSUM") as pspv_pool,
                tc.tile_pool(name="psC", bufs=1, space="PSUM") as psc_pool,
                tc.tile_pool(name="psBC", bufs=1, space="PSUM") as psbc_pool,
            ):
                wpt_sb = wpt_pool.tile([128, CT, C], BF16, tag="wpt")
                for kk in range(CT):
                    nc.sync.dma_start(out=wpt_sb[:, kk, :], in_=wpt_r[:, kk, :])

                # pipeline state
                ps_ss = {}      # gp -> scores PSUM tile
                expts = {}      # gp -> [expt_mt0, expt_mt1, expt_mt2]
                pvs = {}        # gp -> PV PSUM tile
                outts = {}      # b -> outT tile
                pvstAs = {}     # b -> even-head pv staging (+sum row 64)
                pvstBs = {}     # b -> odd-head pv staging (+sum row 63)
                denses = {}     # b -> dense sums tile
                densebs = {}    # b -> bf16 reciprocal tile
                dpalls = {}     # b -> staged recips at partitions 0:2
                proj_ps = {}    # (b, o) -> proj PSUM tile

                def emit_scores(gp, mt):
                    b, p = divmod(gp, 8)
                    o = p
                    moff, msize = M_TILES[mt]
                    mb = (b % 4) * 32 if mt == 2 else 0
                    if mt == 0:
                        ps_ss[gp] = pss_pool.tile(
                            [128, 2, 512], F32, tag="ps_s", name="ps_s"
                        )
                        expts[gp] = [
                            expt_pool.tile(
                                [128, 2, N], BF16, tag=f"expt{m}", name=f"expt{m}"
                            )
                            for m in range(3)
                        ]
                    ps_s = ps_ss[gp]
                    for hh in range(2):
                        rb = 64 * hh
                        nc.tensor.matmul(
                            ps_s[mb : mb + msize, hh, 0:N],
                            lhsT=kt_sb[
                                rb : rb + 64, o, b * N + moff : b * N + moff + msize
                            ],
                            rhs=qt_sb[rb : rb + 64, o, b * N : (b + 1) * N],
                            start=True,
                            stop=True,
                            tile_position=(rb, mb) if mt == 2 else None,
                        )

                def emit_exp(gp, mt):
                    b, p = divmod(gp, 8)
                    moff, msize = M_TILES[mt]
                    mb = (b % 4) * 32 if mt == 2 else 0
                    nc.scalar.activation(
                        out=expts[gp][mt][mb : mb + msize, 0:2, :],
                        in_=ps_ss[gp][mb : mb + msize, :, 0:N],
                        func=mybir.ActivationFunctionType.Exp,
                        bias=zbias_sb[0:msize, 0:1],
                        scale=SCALE,
                    )

                def emit_mask(gp):
                    e0 = expts[gp][0]
                    nc.gpsimd.tensor_tensor(
                        e0[0:32, 0:2, :],
                        e0[0:32, 0:2, :],
                        mask_sb[:, None, :].to_broadcast((32, 2, N)),
                        mybir.AluOpType.mult,
                    )

                def emit_pv(gp, mt):
                    b, p = divmod(gp, 8)
                    moff, msize = M_TILES[mt]
                    mb = (b % 4) * 32 if mt == 2 else 0
                    if mt == 0:
                        pvs[gp] = pspv_pool.tile(
                            [128, 2, 512], F32, tag="ps_pv", name="ps_pv"
                        )
                    ps_pv = pvs[gp]
                    for hh in range(2):
                        if mt < 2:
                            lhsT_v = vaug_sb[
                                0:msize, b, mt, p, 64 * hh : 64 * hh + 128
                            ]
                        else:
                            lhsT_v = v2aug_sb[
                                mb : mb + 32, b // 4, p, 64 * hh : 64 * hh + 128
                            ]
                        nc.tensor.matmul(
                            ps_pv[0:128, hh, 0:N],
                            lhsT=lhsT_v,
                            rhs=expts[gp][mt][mb : mb + msize, hh, :],
                            start=(mt == 0),
                            stop=(mt == 2),
                            skip_group_check=True,
                            tile_position=(mb, 0) if mt == 2 else None,
                        )

                def emit_pv_drain(gp):
                    # PV rows out of PSUM (frees the banks); the softmax-sum
                    # rows (bank0 row 64 / bank1 row 63) ride along, then two
                    # partition-shift DMAs compact them into dense.
                    b, p = divmod(gp, 8)
                    ps_pv = pvs.pop(gp)
                    pvA = pvstAs[b]
                    pvB = pvstBs[b]
                    nc.vector.tensor_copy(pvA[0:65, p, :], ps_pv[0:65, 0, 0:N])
                    nc.vector.tensor_copy(pvB[0:128, p, :], ps_pv[0:128, 1, 0:N])
                    dense = denses[b]
                    nc.sync.dma_start(
                        out=dense[2 * p : 2 * p + 1, :], in_=pvA[64:65, p, :]
                    )
                    nc.sync.dma_start(
                        out=dense[2 * p + 1 : 2 * p + 2, :],
                        in_=pvB[63:64, p, :],
                    )

                def emit_boundary(b):
                    emit_recip(b)

                def emit_recip(b):
                    dense = denses[b]
                    densef = dense_pool.tile(
                        [16, N], F32, tag="densef", name="densef"
                    )
                    denseb = dense_pool.tile(
                        [16, N], BF16, tag="denseb", name="denseb"
                    )
                    densebs[b] = denseb
                    nc.vector.tensor_copy(densef[:], dense[:])
                    nc.vector.reciprocal(out=densef[:], in_=densef[:])
                    nc.vector.tensor_copy(denseb[:], densef[:])

                def emit_bcast_norm(b, o):
                    dp = dp_pool.tile([2, N], BF16, tag="dp", name="dp")
                    nc.sync.dma_start(
                        out=dp[:], in_=densebs[b][2 * o : 2 * o + 2, :]
                    )
                    psbc = psbc_pool.tile([128, N], F32, tag="psbc", name="psbc")
                    nc.tensor.matmul(
                        psbc[:], lhsT=sel2_sb[:], rhs=dp[:], start=True, stop=True
                    )
                    nc.vector.tensor_tensor(
                        outts[b][0:64, o, :],
                        pvstAs[b][0:64, o, :],
                        psbc[0:64, :],
                        mybir.AluOpType.mult,
                    )
                    nc.vector.tensor_tensor(
                        outts[b][64:128, o, :],
                        pvstBs[b][64:128, o, :],
                        psbc[64:128, :],
                        mybir.AluOpType.mult,
                    )

                def emit_proj(b, o, kks, alt=False):
                    if kks[0] == 0:
                        if alt:
                            # epilogue: borrow a scores PSUM buffer so
                            # back-to-back chunks don't serialize on the
                            # single psy bank's WAR
                            t = pss_pool.tile(
                                [128, 2, 512], F32, tag="ps_s", name="ps_s_ep"
                            )
                            proj_ps[(b, o)] = t[:, 0, 0:N]
                        else:
                            proj_ps[(b, o)] = psc_pool.tile(
                                [128, N], F32, tag="psy", name="psy"
                            )
                    ps = proj_ps[(b, o)]
                    for kk in kks:
                        nc.tensor.matmul(
                            ps[:],
                            lhsT=wpt_sb[:, kk, o * 128 : (o + 1) * 128],
                            rhs=outts[b][:, kk, :],
                            start=(kk == 0),
                            stop=(kk == CT - 1),
                        )

                y2_state = {}

                def emit_proj_finish(b, o):
                    # bias-add into a 2-chunk staging tile; one out-DMA per
                    # pair of chunks (halves the sync-queue dispatch load)
                    ps = proj_ps.pop((b, o))
                    if o % 2 == 0:
                        y2_state["t"] = yc_pool.tile(
                            [128, 2, N], F32, tag="y2", name="y2"
                        )
                    y2 = y2_state["t"]
                    nc.vector.tensor_scalar(
                        out=y2[:, o % 2, :],
                        in0=ps[:],
                        scalar1=bp_sb[:, o : o + 1],
                        scalar2=None,
                        op0=mybir.AluOpType.add,
                    )
                    if o % 2 == 1:
                        nc.sync.dma_start(
                            out=out_r[:, o - 1 : o + 1, b * N : (b + 1) * N],
                            in_=y2[:],
                        )

                def new_batch(b):
                    outts[b] = outt_pool.tile(
                        [128, CT, N], BF16, tag="outt", name="outt"
                    )
                    pvstAs[b] = pvst_pool.tile(
                        [128, 8, N], BF16, tag="pvstA", name="pvstA"
                    )
                    pvstBs[b] = pvst_pool.tile(
                        [128, 8, N], BF16, tag="pvstB", name="pvstB"
                    )
                    denses[b] = dense_pool.tile(
                        [16, N], BF16, tag="dense", name="dense"
                    )

                # ---------------- main pipelined loop ----------------
                pv_fifo = []  # gps with scores emitted, PV not yet emitted

                def pop_pv01():
                    g = pv_fifo.pop(0)
                    emit_pv(g, 0)
                    emit_pv(g, 1)
                    return g

                def finish_pv(g):
                    emit_pv(g, 2)
                    emit_pv_drain(g)

                # batches 0-1: no proj filler exists yet, so the per-pair
                # scores->exp PSUM ping-pong would serialize PE<->ACT.
                # Interleave TWO pairs per step (PV lags by 2) to keep both
                # engines streaming; cadence is then ACT(exp)-bound.
                for b in range(2):
                    new_batch(b)
                    for k in range(4):
                        ga, gb = 8 * b + 2 * k, 8 * b + 2 * k + 1
                        emit_scores(ga, 0)
                        emit_scores(gb, 0)
                        # PV of the lag-2 pairs BEFORE the exps: the exps
                        # recycle those pairs' expt buffers (bufs=2), so
                        # emitting them first avoids a WAR stall on ACT
                        pva = pop_pv01() if pv_fifo else None
                        pvb = pop_pv01() if pv_fifo else None
                        emit_exp(ga, 0)
                        emit_mask(ga)
                        emit_exp(gb, 0)
                        emit_mask(gb)
                        emit_scores(ga, 1)
                        emit_exp(ga, 1)
                        if pva is not None:
                            finish_pv(pva)
                        emit_scores(gb, 1)
                        emit_exp(gb, 1)
                        if pvb is not None:
                            finish_pv(pvb)
                            if b == 1 and k == 0:
                                emit_boundary(0)
                        emit_scores(ga, 2)
                        emit_exp(ga, 2)
                        if b == 1 and k >= 1:
                            emit_bcast_norm(0, 2 * (k - 1))
                        emit_scores(gb, 2)
                        emit_exp(gb, 2)
                        if b == 1 and k >= 1:
                            emit_bcast_norm(0, 2 * k - 1)
                        pv_fifo.extend([ga, gb])

                # batches 2-7: steady state, PV lags by 1, proj of batch b-2
                # and the b-1 broadcast/normalize fill the exp latency.
                for gp in range(16, 64):
                    b, p = divmod(gp, 8)
                    if p == 0:
                        new_batch(b)
                    emit_scores(gp, 0)
                    emit_exp(gp, 0)
                    emit_mask(gp)
                    g1 = pop_pv01()
                    emit_proj(b - 2, p, [0, 1, 2])
                    emit_scores(gp, 1)
                    emit_exp(gp, 1)
                    finish_pv(g1)
                    if p == 0:
                        while pv_fifo:  # lag-2 remnant entering batch 2
                            g2 = pop_pv01()
                            finish_pv(g2)
                        emit_boundary(b - 1)
                        if gp == 16:  # batch 0's last two normalizes
                            emit_bcast_norm(0, 6)
                            emit_bcast_norm(0, 7)
                    emit_proj(b - 2, p, [3, 4, 5, 6])
                    if p >= 1:
                        emit_bcast_norm(b - 1, p - 1)
                    emit_scores(gp, 2)
                    emit_exp(gp, 2)
                    emit_proj(b - 2, p, [7])
                    emit_proj_finish(b - 2, p)
                    if p == 7:
                        emit_bcast_norm(b - 1, 7)
                    pv_fifo.append(gp)

                # ---------------- epilogue ----------------
                g = pop_pv01()
                finish_pv(g)
                emit_boundary(7)
                for o in range(CT):
                    emit_proj(6, o, [0, 1, 2, 3], alt=(o % 2 == 1))
                    if o >= 1:
                        emit_bcast_norm(7, o - 1)
                    emit_proj(6, o, [4, 5, 6, 7])
                    emit_proj_finish(6, o)
                emit_bcast_norm(7, 7)
                for o in range(CT):
                    emit_proj(7, o, list(range(CT)), alt=(o % 2 == 1))
                    emit_proj_finish(7, o)

    if split_waits:
        _split_multi_waits(nc)
    return nc


_NC_CACHE = None


def _get_nc():
    global _NC_CACHE
    if _NC_CACHE is None:
        _NC_CACHE = _build_nc()
    return _NC_CACHE


def _host_inputs(x, Wqkv, bqkv, Wproj, bproj):
    bf16 = ml_dtypes.bfloat16
    shared = {}
    shared["wqkt"] = np.ascontiguousarray(Wqkv[: 2 * C].T).astype(bf16)
    shared["wvt"] = np.ascontiguousarray(Wqkv[2 * C :].T).astype(bf16)
    shared["wpt"] = np.ascontiguousarray(Wproj.T).astype(bf16)
    shared["bv"] = bqkv[2 * C :].reshape(1, C).astype(bf16)
    shared["bqk"] = np.ascontiguousarray(
        bqkv[: 2 * C].reshape(2, 8, 128).transpose(2, 0, 1).reshape(128, 16)
    ).astype(np.float32)
    shared["bp"] = np.ascontiguousarray(bproj.reshape(CT, 128).T).astype(np.float32)
    m_ = np.arange(32)[:, None]
    n_ = np.arange(N)[None, :]
    shared["binmask"] = ((n_ < 32) & (n_ >= 4 * (m_ // 4))).astype(bf16)
    sel2 = np.zeros((2, 128), bf16)
    sel2[0, 0:64] = 1.0
    sel2[1, 64:128] = 1.0
    shared["sel2"] = sel2

    in_maps = []
    for i in range(8):
        xc = x[:, i * BL : (i + 1) * BL, :]  # (N, BL, C)
        xt = np.ascontiguousarray(xc.transpose(2, 1, 0).reshape(C, T)).astype(bf16)
        m = dict(shared)
        m["xt"] = xt
        in_maps.append(m)
    return in_maps


def kernel(x, Wqkv, bqkv, Wproj, bproj):
    x = np.asarray(x, dtype=np.float32)
    Wqkv = np.asarray(Wqkv, dtype=np.float32)
    bqkv = np.asarray(bqkv, dtype=np.float32)
    Wproj = np.asarray(Wproj, dtype=np.float32)
    bproj = np.asarray(bproj, dtype=np.float32)

    nc = _get_nc()
    in_maps = _host_inputs(x, Wqkv, bqkv, Wproj, bproj)
    res = run_bass_kernel_spmd(nc, in_maps, core_ids=list(range(8)))

    full = np.empty((N, 64, C), dtype=np.float32)
    for i in range(8):
        yT = np.asarray(res.results[i]["out"], dtype=np.float32)  # [C, T]
        full[:, i * BL : (i + 1) * BL, :] = yT.reshape(C, BL, N).transpose(2, 1, 0)
    return full
